# revision 24
# baseline (speedup 1.0000x reference)
"""Distributed MoE layer (16384 tokens, hidden 1024, ffn 4096, 8 experts, top-2)
on 8 TRN2 NeuronCores — FFN-tensor-parallel for perfect load balance.

Host: router (same semantics as the jax reference) + expert-sort of token
slots + final combine (sum of 8 partial-FFN outputs, scatter-add to tokens).
Device (core c): for every routed token slot, computes its expert's FFN
restricted to ffn columns [c*512, (c+1)*512), in bf16 with fp32 PSUM
accumulation:  y_partial = coeff * (gelu(x @ w1[e][:, sl]) @ w2[e][sl, :]).
Every core processes the same (expert-sorted, 128-padded) token stream, so
the PE work per core is identical regardless of how uneven the routing is.
"""

import sys

if "/opt/trn_rl_repo" not in sys.path:
    sys.path.insert(0, "/opt/trn_rl_repo")

import numpy as np
import ml_dtypes

import concourse.mybir as mybir
import concourse.tile as tile
from concourse import bacc
from concourse.bass_utils import run_bass_kernel_spmd

N_TOKENS = 16384
HIDDEN = 1024
FFN = 4096
N_EXPERTS = 8
TOP_K = 2
P = 128
N_CORES = 8
FL = FFN // N_CORES  # 512: ffn slice per core
FLC = FL // P  # 4 chunks of the local ffn slice
KH = HIDDEN // P  # 8 hidden chunks
TG = 512  # token group (moving-operand width)

BF16 = mybir.dt.bfloat16
FP32 = mybir.dt.float32
NPBF16 = ml_dtypes.bfloat16

_KERNEL_CACHE: dict[tuple, object] = {}


def _build(true_counts: tuple[int, ...]):
    """One SPMD program: all cores process the same token-slot stream.

    true_counts[e] = number of token slots routed to expert e; its segment in
    the slot stream is padded to a multiple of 128. GEMM1 streams only the
    true widths; GEMM2/y are 128-chunk quantized (pad rows compute garbage
    that the host discards).
    """
    segs = tuple(-(-c // P) * P for c in true_counts)
    S = sum(segs)
    NCHUNK = S // P
    nc = bacc.Bacc("TRN2", target_bir_lowering=False, debug=False)
    gelu = mybir.ActivationFunctionType.Gelu_apprx_tanh

    with tile.TileContext(nc) as tc:
        with tc.tile_pool(name="dram", bufs=1, space="DRAM") as dram:
            xt = dram.tile([HIDDEN, S], BF16, kind="ExternalInput", uniquify=False, name="xt")
            w1l = dram.tile(
                [P, N_EXPERTS * FLC * KH, P], BF16, kind="ExternalInput", uniquify=False, name="w1l"
            )
            w2l = dram.tile(
                [P, N_EXPERTS * FLC, HIDDEN], BF16, kind="ExternalInput", uniquify=False, name="w2l"
            )
            cf = dram.tile([P, NCHUNK], FP32, kind="ExternalInput", uniquify=False, name="cf")
            y = dram.tile([S, HIDDEN], BF16, kind="ExternalOutput", uniquify=False, name="y")
            # transposed output for pad-free (token-moving) GEMM2 groups;
            # the combine weight for these slots is applied on the host
            yT = dram.tile([HIDDEN, S], BF16, kind="ExternalOutput", uniquify=False, name="yT")

            xt3 = xt[:].rearrange("(ko p) n -> p ko n", p=P)  # [128, KH, S]
            yT3 = yT[:].rearrange("(hc p) n -> p hc n", p=P)  # [128, KH, S]

            with tc.tile_pool(name="wp", bufs=1) as wp, tc.tile_pool(
                name="cp", bufs=1
            ) as cp, tc.tile_pool(name="xp", bufs=3) as xp, tc.tile_pool(
                name="hp", bufs=2
            ) as hp, tc.tile_pool(name="yp", bufs=2) as yp, tc.tile_pool(
                name="pp1", bufs=2, space="PSUM"
            ) as pp1, tc.tile_pool(name="pp2", bufs=3, space="PSUM") as pp2:
                w1sb = wp.tile([P, N_EXPERTS * FLC * KH, P], BF16, name="w1sb")
                w2sb = wp.tile([P, N_EXPERTS * FLC, HIDDEN], BF16, name="w2sb")
                cfsb = cp.tile([P, NCHUNK], FP32, name="cfsb")

                # weight-load thunks, one per 256KB chunk, in first-use order.
                # Issued on the SCALAR engine, interleaved after each group's
                # gelus: the ACT stream is paced by the compute pipeline, so
                # the 33.6MB weight stream trickles in instead of flooding the
                # DMA queues and starving the x stream (v1/v2 had a ~20us PE
                # stall from exactly that: gpsimd has no deps on the group
                # loop, so gpsimd-issued weight DMAs all fire at t=0).
                def wdma_thunks(e, eng):
                    ts = []
                    for fl in range(FLC):
                        c0 = (e * FLC + fl) * KH
                        ts.append(
                            lambda c0=c0: eng.dma_start(
                                w1sb[:, c0 : c0 + KH, :], w1l[:, c0 : c0 + KH, :]
                            )
                        )
                    for fc in range(FLC):
                        c0 = e * FLC + fc
                        ts.append(
                            lambda c0=c0: eng.dma_start(w2sb[:, c0, :], w2l[:, c0, :])
                        )
                    return ts

                # token groups: (expert, global slot offset, true width).
                # A tail group narrower than 257 tokens is LDWEIGHTS-bound on
                # GEMM1 (107ns load > w/2.4 stream), so rebalance the last two
                # groups, keeping every group start 128-aligned (cfsb/y-store
                # indexing needs that) and the total 128-chunk count unchanged.
                groups = []
                off = 0
                for e, cnt in enumerate(true_counts):
                    ws = []
                    rem = cnt
                    while rem > TG + 256:
                        ws.append(TG)
                        rem -= TG
                    if rem <= TG:
                        ws.append(rem)
                    else:
                        w = rem - TG  # in (0, 256]
                        b = w + 256 if w <= 128 else w + 128
                        ws.append(TG + w - b)  # 256 or 384
                        ws.append(b)  # in [257, 384]
                    p0 = 0
                    for w in ws:
                        groups.append((e, off + p0, w))
                        p0 += w
                    off += segs[e]

                # expert 0's weights + cf upfront; later experts' weights are
                # paced one chunk per group on gpsimd, where each chunk is
                # gated by a tiny copy that reads the current group's hg: the
                # DMA issue then cannot start before the group is in flight,
                # and if the descriptor ring backpressures, only idle gpsimd
                # blocks (a blocked issue ahead of the gelus/muls stalls the
                # PSUM pipeline and with it the PE — v3's remaining gaps).
                e0_thunks = wdma_thunks(0, nc.gpsimd)
                # e0's w1 fl0 goes per-k-chunk (8 x 32KB): the very first
                # matmul then depends on only 160KB of DMA (xg k0 + this), so
                # the PE starts ~7us in at the cold 1.2GHz rate -- the cold
                # phase doubles the DMA slack and acts as the HAM warmup.
                for k in range(KH):
                    nc.gpsimd.dma_start(w1sb[:, k, :], w1l[:, k, :])
                for t in e0_thunks[1:FLC]:  # e0's w1 fl1-3
                    t()
                nc.gpsimd.dma_start(cfsb[:], cf[:])
                # ~3.5us gpsimd scratch memset before e0's w2 enters the
                # queues: its 1MB is not needed until GEMM2(g0) (~t=16us),
                # and issuing it at t=0 puts its descriptors ahead of group
                # 0/1's x stream. A pure time delay on idle gpsimd (nop
                # lowering is unimplemented here) has no dependencies, so it
                # cannot feed stalls back into any pipeline the way gated
                # issues did.
                dly = cp.tile([P, 4096], BF16, name="dly")
                nc.gpsimd.memset(dly[:], 0.0)
                for t in e0_thunks[FLC:]:  # e0's w2
                    t()
                wqueue = []  # (expert, thunk) still to issue
                for e in range(1, N_EXPERTS):
                    for t in wdma_thunks(e, nc.gpsimd):
                        wqueue.append((e, t))
                pacet = cp.tile([P, 1], BF16, name="pacet")

                def load_x(gi):
                    _, g0, w = groups[gi]
                    xg = xp.tile([P, KH, TG], BF16, name="xg", tag="xg")
                    if gi <= 2:
                        # split per k-chunk so the k=0 matmul starts earliest
                        # and chunks arrive in consumption order; alternate the
                        # two HWDGE rings (sync + scalar) to double the
                        # early-stream DMA issue throughput
                        for k in range(KH):
                            eng = nc.sync if k % 2 == 0 else nc.scalar
                            eng.dma_start(xg[:, k, :w], xt3[:, k, g0 : g0 + w])
                    else:
                        nc.sync.dma_start(xg[:, :, :w], xt3[:, :, g0 : g0 + w])
                    return xg

                xgs = {}
                npre = min(3, len(groups))
                for gi in range(npre):
                    xgs[gi] = load_x(gi)

                for gi, (e, g0, w) in enumerate(groups):
                    xg = xgs.pop(gi)
                    if gi + npre < len(groups):
                        xgs[gi + npre] = load_x(gi + npre)
                    # ---- GEMM1: h = gelu(w1_chunk.T @ x), kept in SBUF ----
                    hg = hp.tile([P, FLC, TG], BF16, name="hg", tag="hg")
                    if w % P:
                        # pad columns up to the next 128 boundary feed GEMM2's
                        # discarded pad rows; zero them so the first rotations
                        # never read junk (beyond nt*P GEMM2 never reads)
                        nc.vector.memset(hg[:, :, w : -(-w // P) * P], 0.0)
                    for fl in range(FLC):
                        ps = pp1.tile([P, TG], FP32, name="ps", tag="ps")
                        ci = (e * FLC + fl) * KH
                        for k in range(KH):
                            nc.tensor.matmul(
                                ps[:, :w],
                                lhsT=w1sb[:, ci + k, :],
                                rhs=xg[:, k, :w],
                                start=(k == 0),
                                stop=(k == KH - 1),
                            )
                        nc.scalar.activation(hg[:, fl, :w], ps[:, :w], gelu)

                    # ---- GEMM2 ----
                    last = gi == len(groups) - 1
                    if w % P == 0:
                        # token-stationary form: y = coeff * (h.T @ w2) with
                        # w2 moving (512 cols); zero pad waste since w%128==0
                        nt = w // P
                        yt = yp.tile([P, TG // P, HIDDEN], BF16, name="yt", tag="yt")
                        for t in range(nt):
                            psA = pp2.tile([P, TG], FP32, name="psA", tag="psA")
                            psB = pp2.tile([P, TG], FP32, name="psB", tag="psB")
                            for fc in range(FLC):
                                lt = hg[:, fc, t * P : (t + 1) * P]
                                nc.tensor.matmul(
                                    psA[:],
                                    lhsT=lt,
                                    rhs=w2sb[:, e * FLC + fc, 0:TG],
                                    start=(fc == 0),
                                    stop=(fc == FLC - 1),
                                )
                                nc.tensor.matmul(
                                    psB[:],
                                    lhsT=lt,
                                    rhs=w2sb[:, e * FLC + fc, TG:HIDDEN],
                                    start=(fc == 0),
                                    stop=(fc == FLC - 1),
                                )
                            ct = cfsb[:, g0 // P + t : g0 // P + t + 1]
                            nc.vector.tensor_scalar_mul(yt[:, t, 0:TG], psA[:], ct)
                            nc.vector.tensor_scalar_mul(yt[:, t, TG:HIDDEN], psB[:], ct)
                            if last:
                                # tail latency: ship each 128-row chunk as soon
                                # as its PSUM is evacuated, not one 1MB DMA
                                ydt = y[g0 + t * P : g0 + (t + 1) * P, :].rearrange(
                                    "(t p) h -> p t h", p=P
                                )
                                nc.sync.dma_start(ydt, yt[:, t : t + 1, :])
                        if not last:
                            ydst = y[g0 : g0 + nt * P, :].rearrange(
                                "(t p) h -> p t h", p=P
                            )
                            nc.sync.dma_start(ydst, yt[:, :nt, :])
                    else:
                        # token-moving form for ragged tails: yT = w2.T @ h
                        # streams exactly w columns (no 128-pad rows), written
                        # transposed; host applies the combine weight. hc pairs
                        # interleave so hg[fl=3] (last gelu) isn't needed until
                        # MM #7 of the group, hiding the gelu latency.
                        ytT = yp.tile([P, KH, TG], BF16, name="ytT", tag="yt")
                        for hcp in range(0, KH, 2):
                            psA = pp2.tile([P, TG], FP32, name="psA", tag="psA")
                            psB = pp2.tile([P, TG], FP32, name="psB", tag="psB")
                            for fc in range(FLC):
                                wc = e * FLC + fc
                                nc.tensor.matmul(
                                    psA[:, :w],
                                    lhsT=w2sb[:, wc, hcp * P : (hcp + 1) * P],
                                    rhs=hg[:, fc, :w],
                                    start=(fc == 0),
                                    stop=(fc == FLC - 1),
                                )
                                nc.tensor.matmul(
                                    psB[:, :w],
                                    lhsT=w2sb[:, wc, (hcp + 1) * P : (hcp + 2) * P],
                                    rhs=hg[:, fc, :w],
                                    start=(fc == 0),
                                    stop=(fc == FLC - 1),
                                )
                            nc.vector.tensor_copy(ytT[:, hcp, :w], psA[:, :w])
                            nc.vector.tensor_copy(ytT[:, hcp + 1, :w], psB[:, :w])
                            if last:
                                nc.sync.dma_start(
                                    yT3[:, hcp : hcp + 2, g0 : g0 + w],
                                    ytT[:, hcp : hcp + 2, :w],
                                )
                        if not last:
                            nc.sync.dma_start(
                                yT3[:, :, g0 : g0 + w], ytT[:, :, :w]
                            )

                    # pace the weight stream: expert e+1's chunks must all be
                    # issued by the end of expert e's groups; anything overdue
                    # (empty-segment experts) flushes immediately. The gate is
                    # a 1-descriptor read-back of a y row this group just
                    # stored: a pure DRAM dependency, so a ring-blocked issue
                    # can only ever delay later weight chunks, never the
                    # compute pipeline (scalar/vector/hg-gated variants all
                    # fed stalls back into the PE).
                    ngleft = sum(1 for ee, _, _ in groups[gi:] if ee == e)
                    nw = sum(1 for ee, _ in wqueue if ee == e + 1)
                    nover = sum(1 for ee, _ in wqueue if ee <= e)
                    nissue = nover + (0 if nw == 0 else max(1, -(-nw // ngleft)))
                    if nissue and wqueue:
                        if w % P == 0:
                            nc.gpsimd.dma_start(pacet[0:1, 0:1], y[g0 : g0 + 1, 0:1])
                        else:
                            nc.gpsimd.dma_start(pacet[0:1, 0:1], yT[0:1, g0 : g0 + 1])
                        for _ in range(nissue):
                            if wqueue:
                                wqueue.pop(0)[1]()

    nc.compile()
    return nc


def _get_kernel(counts: tuple[int, ...]):
    if counts not in _KERNEL_CACHE:
        _KERNEL_CACHE[counts] = _build(counts)
    return _KERNEL_CACHE[counts]


def _route(x: np.ndarray, w_router: np.ndarray):
    """Replicates the reference router: softmax -> top-2 -> renormalize."""
    logits = x @ w_router  # [N, E] fp32
    order = np.argsort(-logits, axis=1, kind="stable")
    i1, i2 = order[:, 0], order[:, 1]
    l64 = logits.astype(np.float64)
    l64 -= l64.max(axis=1, keepdims=True)
    e = np.exp(l64)
    p = e / e.sum(axis=1, keepdims=True)
    rows = np.arange(x.shape[0])
    p1 = p[rows, i1]
    p2 = p[rows, i2]
    s = p1 + p2
    return i1, i2, (p1 / s).astype(np.float32), (p2 / s).astype(np.float32)


def _group_widths(cnt):
    """Mirrors the kernel's per-expert group splitting."""
    ws = []
    rem = cnt
    while rem > TG + 256:
        ws.append(TG)
        rem -= TG
    if rem <= TG:
        ws.append(rem)
    else:
        w = rem - TG
        b = w + 256 if w <= 128 else w + 128
        ws.append(TG + w - b)
        ws.append(b)
    return ws


def prepare(x, w_router, w1, w2):
    """Host routing + input staging. Returns (counts, in_maps, tok_e, seg_off, counts_arr, cfs, ytr)."""
    x = np.ascontiguousarray(x, dtype=np.float32)
    w_router = np.ascontiguousarray(w_router, dtype=np.float32)
    w1 = np.ascontiguousarray(w1, dtype=np.float32)
    w2 = np.ascontiguousarray(w2, dtype=np.float32)
    n = x.shape[0]

    i1, i2, c1, c2 = _route(x, w_router)

    slot_expert = np.concatenate([i1, i2])
    slot_coeff = np.concatenate([c1, c2])
    slot_token = np.concatenate([np.arange(n), np.arange(n)])
    counts = np.bincount(slot_expert, minlength=N_EXPERTS)

    order = np.argsort(slot_expert, kind="stable")
    tok_sorted = slot_token[order]
    coef_sorted = slot_coeff[order]
    starts = np.concatenate([[0], np.cumsum(counts)])

    segs = tuple(int(-(-c // P) * P) for c in counts)
    S = sum(segs)
    seg_off = np.concatenate([[0], np.cumsum(segs)])

    xs = np.zeros((S, HIDDEN), dtype=np.float32)
    cfs = np.zeros(S, dtype=np.float32)
    tok_e = []
    for e in range(N_EXPERTS):
        te = tok_sorted[starts[e] : starts[e + 1]]
        tok_e.append(te)
        xs[seg_off[e] : seg_off[e] + len(te)] = x[te]
        cfs[seg_off[e] : seg_off[e] + len(te)] = coef_sorted[starts[e] : starts[e + 1]]

    xt = np.ascontiguousarray(xs.T).astype(NPBF16)  # [HIDDEN, S]
    cf2 = np.ascontiguousarray(cfs.reshape(S // P, P).T)  # [P, S//P]

    # slot ranges handled by the token-moving GEMM2 (output in yT, transposed,
    # without the combine weight applied)
    ytr = []
    for e in range(N_EXPERTS):
        p0 = 0
        for w in _group_widths(int(counts[e])):
            if w % P:
                ytr.append((int(seg_off[e] + p0), w))
            p0 += w

    w1b = w1.astype(NPBF16)
    w2b = w2.astype(NPBF16)
    in_maps = []
    for c in range(N_CORES):
        base = c * FL
        w1s = (
            w1b[:, :, base : base + FL]
            .reshape(N_EXPERTS, KH, P, FLC, P)
            .transpose(2, 0, 3, 1, 4)
            .reshape(P, N_EXPERTS * FLC * KH, P)
        )
        w2s = (
            w2b[:, base : base + FL, :]
            .reshape(N_EXPERTS, FLC, P, HIDDEN)
            .transpose(2, 0, 1, 3)
            .reshape(P, N_EXPERTS * FLC, HIDDEN)
        )
        in_maps.append(
            {
                "xt": xt,
                "w1l": np.ascontiguousarray(w1s),
                "w2l": np.ascontiguousarray(w2s),
                "cf": cf2,
            }
        )
    return tuple(int(c) for c in counts), in_maps, tok_e, seg_off, counts, cfs, ytr


def combine(results, tok_e, seg_off, n, cfs, ytr):
    acc = results[0]["y"].astype(np.float32)
    for c in range(1, N_CORES):
        acc += results[c]["y"].astype(np.float32)
    for r0, w in ytr:
        accT = results[0]["yT"][:, r0 : r0 + w].astype(np.float32)
        for c in range(1, N_CORES):
            accT += results[c]["yT"][:, r0 : r0 + w].astype(np.float32)
        acc[r0 : r0 + w] = cfs[r0 : r0 + w, None] * accT.T
    out = np.zeros((n, HIDDEN), dtype=np.float32)
    for e in range(N_EXPERTS):
        te = tok_e[e]
        out[te] += acc[seg_off[e] : seg_off[e] + len(te)]
    return out


def kernel(x, w_router, w1, w2):
    counts, in_maps, tok_e, seg_off, _, cfs, ytr = prepare(x, w_router, w1, w2)
    nc = _get_kernel(counts)
    res = run_bass_kernel_spmd(nc, in_maps, core_ids=list(range(N_CORES)))
    return combine(res.results, tok_e, seg_off, x.shape[0], cfs, ytr)

